# revision 10
# baseline (speedup 1.0000x reference)
"""Trainium2 Bass kernel for nn_CFGEncoder (3-layer directed GCN + BN + pool).

Self-contained: accepts FULL inputs, shards across 8 NeuronCores internally,
returns the FULL [64, 128] output.

Strategy (per layer, per direction):
  - nodes sharded 12500/core (by scatter target); edges partitioned accordingly
  - gathered-node features fetched with GPSIMD dma_gather (int16 indices into
    4 chunk tables of 25000 rows each)
  - segment-sum realized as one-hot matmuls on PE: for each 128-edge tile,
    psum[d, 128nodes] += gathered[128e, d].T @ onehot[128e, 128]
  - dense part z = Ws.T@xT + Wi.T@aggin + Wo.T@aggout on PE (fp32),
    relu+batchnorm stats on ACT, global stats via AllReduce
  - node features exchanged between layers with an 8-core AllGather
  - graph mean-pool via one-hot matmul + AllReduce
"""
import sys

sys.path.insert(0, "/opt/trn_rl_repo")

import numpy as np

NCORE = 8
NCHUNK = 4
G = 128            # scatter window width (nodes per PSUM group)
BT = 4             # 128-edge tiles per dma_gather call (512 idx fits SWDGE ring)
NQ = 4             # SWDGE queues, rotated per gather call
STRIPE = 512       # dense-phase node stripe
BN_EPS = 1e-5
NUM_GRAPHS = 64


# ----------------------------------------------------------------------------
# host-side preprocessing
# ----------------------------------------------------------------------------

def _edge_streams(gat, sct, N):
    """Build per-core padded gather-index / scatter-loc streams for one
    direction.  gat = node gathered per edge, sct = scatter target per edge.

    Returns dict with:
      idx_wrapped [NCORE, 128, L//16] int16  (dma_gather index layout)
      loc_tiled   [NCORE, 128, L//128] f32   (one-hot position layout)
      seg_tiles   [NCHUNK, NW] int   (tiles per (chunk, window) segment)
      L           common per-core stream length (multiple of 128)
    """
    S = N // NCORE
    CH = N // NCHUNK
    NW = (S + G - 1) // G
    core = sct // S
    q = gat // CH
    locidx = (gat - q * CH).astype(np.int64)
    w = (sct % S) // G
    lw = (sct % S) % G

    ngrp = NCORE * NCHUNK * NW
    key = (core * NCHUNK + q) * NW + w
    cnt = np.bincount(key, minlength=ngrp).reshape(NCORE, NCHUNK, NW)
    seg_tiles = np.maximum((cnt + 127) // 128, 1).max(axis=0)  # [NCHUNK, NW]
    # per-(q,w) tile counts common to all cores; keep >=1 tile only if any
    # core has edges there; zero-edge-everywhere segments get dropped
    any_edges = cnt.sum(axis=0) > 0
    seg_tiles = np.where(any_edges, seg_tiles, 0)

    seg_len = seg_tiles * 128                                    # [NCHUNK, NW]
    seg_off = np.zeros(NCHUNK * NW + 1, np.int64)
    np.cumsum(seg_len.reshape(-1), out=seg_off[1:])
    L = int(seg_off[-1])
    assert L % 128 == 0

    order = np.argsort(key, kind="stable")
    skey = key[order]
    # rank of each edge within its (core,q,w) group
    grp_start = np.zeros(ngrp + 1, np.int64)
    np.cumsum(np.bincount(skey, minlength=ngrp), out=grp_start[1:])
    rank = np.arange(len(order)) - grp_start[skey]
    pos = seg_off[(q * NW + w)[order]] + rank

    idx_stream = np.zeros((NCORE, L), np.int16)          # pad -> row 0
    loc_stream = np.full((NCORE, L), -1.0, np.float32)   # pad -> no window hit
    idx_stream[core[order], pos] = locidx[order].astype(np.int16)
    loc_stream[core[order], pos] = lw[order].astype(np.float32)

    idxw = idx_stream.reshape(NCORE, L // 16, 16).transpose(0, 2, 1)  # [NC,16,L/16]
    idx_wrapped = np.tile(idxw, (1, 8, 1)).copy()                     # [NC,128,L/16]
    loc_tiled = (
        loc_stream.reshape(NCORE, L // 128, 128).transpose(0, 2, 1).copy()
    )
    return dict(idx_wrapped=idx_wrapped, loc_tiled=loc_tiled,
                seg_tiles=seg_tiles, L=L, NW=NW)


def _preprocess(x, edge_index, batch):
    N, Din = x.shape
    S = N // NCORE
    src = edge_index[0].astype(np.int64)
    dst = edge_index[1].astype(np.int64)
    stream_in = _edge_streams(src, dst, N)    # aggregate x[src] onto dst
    stream_out = _edge_streams(dst, src, N)   # aggregate x[dst] onto src

    xs = np.stack([x[c * S:(c + 1) * S] for c in range(NCORE)])  # [NC, S, Din]

    NT = (S + 127) // 128
    bl = np.full((NCORE, NT * 128), -1.0, np.float32)
    for c in range(NCORE):
        bl[c, :S] = batch[c * S:(c + 1) * S].astype(np.float32)
    bloc = bl.reshape(NCORE, NT, 128).transpose(0, 2, 1).copy()  # [NC,128,NT]

    cnts = np.bincount(batch.astype(np.int64), minlength=NUM_GRAPHS).astype(np.float32)
    recip = (1.0 / np.maximum(cnts, 1.0)).reshape(NUM_GRAPHS, 1)
    return stream_in, stream_out, xs, bloc, recip


# ----------------------------------------------------------------------------
# kernel build
# ----------------------------------------------------------------------------

def _build(nc, tile, mybir, bass, meta):
    """Emit the full 3-layer program into nc (shared by all cores)."""
    N, Din = meta["N"], meta["Din"]
    S = N // NCORE
    CH = N // NCHUNK
    NW = (S + G - 1) // G
    NT = (S + 127) // 128          # node tiles per core slice
    dims = meta["dims"]            # [Din, 128, 128, 128]
    s_in, s_out = meta["s_in"], meta["s_out"]
    AT = mybir.ActivationFunctionType
    f32 = mybir.dt.float32
    i16 = mybir.dt.int16
    from concourse.masks import make_identity
    from concourse import library_config

    # ---- DRAM I/O ----
    xp_d = nc.dram_tensor("xp", [N, Din], f32, kind="ExternalInput")
    xs_d = nc.dram_tensor("xs", [S, Din], f32, kind="ExternalInput")
    idx_d, loc_d = {}, {}
    for dname, st in (("in", s_in), ("out", s_out)):
        idx_d[dname] = nc.dram_tensor(
            f"idx_{dname}", [128, st["L"] // 16], i16, kind="ExternalInput")
        loc_d[dname] = nc.dram_tensor(
            f"loc_{dname}", [128, st["L"] // 128], f32, kind="ExternalInput")
    w_d = {}
    for l in range(3):
        for wn in ("s", "i", "o"):
            w_d[(l, wn)] = nc.dram_tensor(
                f"w{wn}{l}", [dims[l], dims[l + 1]], f32, kind="ExternalInput")
    gb_d = [nc.dram_tensor(f"gb{l}", [dims[l + 1], 2], f32, kind="ExternalInput")
            for l in range(3)]
    bloc_d = nc.dram_tensor("bloc", [128, NT], f32, kind="ExternalInput")
    recip_d = nc.dram_tensor("recip", [NUM_GRAPHS, 1], f32, kind="ExternalInput")
    out_d = nc.dram_tensor("out", [NUM_GRAPHS, 128], f32, kind="ExternalOutput")

    # internal DRAM
    import os as _os
    _shared = "Local" if _os.environ.get("DBG_NO_SHARED") else "Shared"
    hfull = [nc.dram_tensor(f"hfull{l}", [N, 128], f32, addr_space=_shared)
             for l in (1, 2)]                       # gather tables for layers 1,2
    hslice = [nc.dram_tensor(f"hslice{l}", [S, 128], f32) for l in (1, 2)]
    zraw = nc.dram_tensor("zraw", [128, S], f32)
    bn_in = nc.dram_tensor("bn_in", [128, 2], f32)
    bn_out = nc.dram_tensor("bn_out", [128, 2], f32, addr_space=_shared)
    pool_in = nc.dram_tensor("pool_in", [NUM_GRAPHS, 128], f32)
    pool_out = nc.dram_tensor("pool_out", [NUM_GRAPHS, 128], f32, addr_space=_shared)

    with tile.TileContext(nc) as tc:
        nc.gpsimd.load_library(library_config.mlp)
        import contextlib
        ctx = contextlib.ExitStack()
        const = ctx.enter_context(tc.tile_pool(name="const", bufs=1))
        big = ctx.enter_context(tc.tile_pool(name="big", bufs=2))
        gat = ctx.enter_context(tc.tile_pool(name="gat", bufs=3))
        ohp = ctx.enter_context(tc.tile_pool(name="ohp", bufs=3))
        idxp = ctx.enter_context(tc.tile_pool(name="idxp", bufs=1))
        locp = ctx.enter_context(tc.tile_pool(name="locp", bufs=1))
        wrk = ctx.enter_context(tc.tile_pool(name="wrk", bufs=3))
        stg = ctx.enter_context(tc.tile_pool(name="stg", bufs=3))
        scps = ctx.enter_context(tc.tile_pool(name="scps", bufs=3, space="PSUM"))
        dnps = ctx.enter_context(tc.tile_pool(name="dnps", bufs=2, space="PSUM"))
        tpps = ctx.enter_context(tc.tile_pool(name="tpps", bufs=2, space="PSUM"))
        plps = ctx.enter_context(tc.tile_pool(name="plps", bufs=1, space="PSUM"))

        # ---- constants ----
        iota = const.tile([128, G], f32)
        nc.gpsimd.iota(iota[:], pattern=[[1, G]], base=0, channel_multiplier=0,
                       allow_small_or_imprecise_dtypes=True)
        ident = const.tile([128, 128], f32)
        make_identity(nc, ident[:])
        w_sb = {}
        for l in range(3):
            for wn in ("s", "i", "o"):
                t = const.tile([dims[l], dims[l + 1]], f32, name=f"w{wn}{l}_sb")
                nc.sync.dma_start(t[:], w_d[(l, wn)][:])
                w_sb[(l, wn)] = t
        gb_sb = []
        for l in range(3):
            t = const.tile([dims[l + 1], 2], f32, name=f"gb{l}_sb")
            nc.sync.dma_start(t[:], gb_d[l][:])
            gb_sb.append(t)
        recip_sb = const.tile([NUM_GRAPHS, 1], f32)
        nc.sync.dma_start(recip_sb[:], recip_d[:])
        bloc_sb = const.tile([128, NT], f32)
        nc.sync.dma_start(bloc_sb[:], bloc_d[:])
        iota_g = const.tile([128, NUM_GRAPHS], f32)
        nc.gpsimd.iota(iota_g[:], pattern=[[1, NUM_GRAPHS]], base=0,
                       channel_multiplier=0, allow_small_or_imprecise_dtypes=True)

        # ================= scatter phase =================
        qctr = [0]

        def scatter_direction(l, st, idx_dram, loc_dram, src_dram, d_in, agg):
            """Aggregate gathered features into agg [d_in, S] (SBUF)."""
            seg_tiles = st["seg_tiles"]
            nc.vector.memset(agg[:], 0.0)
            import os as _os2
            if _os2.environ.get("DBG_SKIP_SCATTER"):
                return
            _skip_gather = bool(_os2.environ.get("DBG_SKIP_GATHER"))
            _skip_mm = bool(_os2.environ.get("DBG_SKIP_MM"))
            # chunk tile offsets in the stream
            seg_len = seg_tiles * 128
            seg_off = np.zeros(NCHUNK * NW + 1, np.int64)
            np.cumsum(seg_len.reshape(-1), out=seg_off[1:])
            for q in range(NCHUNK):
                t0 = int(seg_off[q * NW]) // 128          # first tile of chunk
                t1 = int(seg_off[(q + 1) * NW]) // 128    # end tile
                ntile = t1 - t0
                if ntile == 0:
                    continue
                # load idx / loc slices for this chunk
                idx_sb = idxp.tile([128, ntile * 8], i16, name=f"idx_sb_l{l}", tag="idx")
                nc.sync.dma_start(idx_sb[:], idx_dram[:, t0 * 8: t1 * 8])
                loc_sb = locp.tile([128, ntile], f32, name=f"loc_sb_l{l}", tag="loc")
                nc.sync.dma_start(loc_sb[:], loc_dram[:, t0: t1])
                src_ap = src_dram[q * CH:(q + 1) * CH, :d_in]
                # gather calls + one-hot builds, batches of BT tiles
                bufs = []   # (first_tile_local, tile_obj, oh_obj, ntiles)
                for b0 in range(0, ntile, BT):
                    nb = min(BT, ntile - b0)
                    gt = gat.tile([128, BT, d_in], f32, name=f"gt_l{l}", tag="gt")
                    nidx = nb * 128
                    if _skip_gather:
                        nc.vector.memset(gt[:, :nb, :], 0.0)
                    else:
                        nc.gpsimd.dma_gather(
                            gt[:, :nb, :], src_ap, idx_sb[:, b0 * 8:(b0 + nb) * 8],
                            nidx, nidx, d_in, queue_num=qctr[0] % NQ)
                        qctr[0] += 1
                    oh = ohp.tile([128, BT, G], f32, name=f"oh_l{l}", tag="oh")
                    nc.vector.tensor_tensor(
                        out=oh[:, :nb, :],
                        in0=loc_sb[:, b0:b0 + nb].unsqueeze(2)
                            .broadcast_to([128, nb, G]),
                        in1=iota[:].unsqueeze(1).broadcast_to([128, nb, G]),
                        op=mybir.AluOpType.is_equal)
                    bufs.append((b0, gt, oh, nb))

                def tile_ref(t):  # local tile index -> (gt, oh, slot)
                    bi = t // BT
                    return bufs[bi][1], bufs[bi][2], t - bufs[bi][0]

                for w in range(NW):
                    ntw = int(seg_tiles[q, w])
                    if ntw == 0:
                        continue
                    base = int(seg_off[q * NW + w]) // 128 - t0
                    gw = min(G, S - w * G)
                    ps = scps.tile([d_in, G], f32, space="PSUM", name=f"scps_l{l}", tag="sc")
                    if _skip_mm:
                        continue
                    for k in range(ntw):
                        gt, oh, slot = tile_ref(base + k)
                        nc.tensor.matmul(
                            out=ps[:], lhsT=gt[:, slot, :], rhs=oh[:, slot, :],
                            start=(k == 0), stop=(k == ntw - 1))
                    nc.vector.tensor_tensor(
                        out=agg[:, w * G: w * G + gw],
                        in0=agg[:, w * G: w * G + gw],
                        in1=ps[:, :gw], op=mybir.AluOpType.add)

        # ============== dense + BN ==============
        def dense_layer(l, d_in, d_out, x_nm_dram, agg_in, agg_out):
            """zraw[d_out, S] = relu(Ws.T xT + Wi.T agg_in + Wo.T agg_out);
            returns per-partition BN (scale, bias) tiles."""
            nstripe = (S + STRIPE - 1) // STRIPE
            stats = wrk.tile([d_out, 2 * nstripe + 8], f32, name=f"stats{l}", tag="stats")
            sq = wrk.tile([d_out, STRIPE], f32, name=f"sq{l}", tag="sq", bufs=2)
            for s in range(nstripe):
                n0 = s * STRIPE
                ns = min(STRIPE, S - n0)
                # build xT stripe [d_in, ns] by transposing node-major rows
                xT = wrk.tile([d_in, STRIPE], f32, name=f"xT{l}", tag="xT")
                for t0 in range(0, ns, 128):
                    tn = min(128, ns - t0)
                    xr = stg.tile([128, d_in], f32, name=f"xr{l}", tag="xr")
                    nc.sync.dma_start(xr[:tn, :], x_nm_dram[n0 + t0: n0 + t0 + tn, :d_in])
                    tp = tpps.tile([d_in, 128], f32, space="PSUM", name=f"xtp{l}", tag="tp")
                    nc.tensor.transpose(out=tp[:, :tn], in_=xr[:tn, :],
                                        identity=ident[:tn, :tn])
                    nc.scalar.copy(xT[:, t0: t0 + tn], tp[:, :tn])
                ps = dnps.tile([d_out, STRIPE], f32, space="PSUM", name=f"dn{l}", tag="dn")
                nc.tensor.matmul(out=ps[:, :ns], lhsT=w_sb[(l, "s")][:],
                                 rhs=xT[:, :ns], start=True, stop=False)
                nc.tensor.matmul(out=ps[:, :ns], lhsT=w_sb[(l, "i")][:],
                                 rhs=agg_in[:, n0:n0 + ns], start=False, stop=False)
                nc.tensor.matmul(out=ps[:, :ns], lhsT=w_sb[(l, "o")][:],
                                 rhs=agg_out[:, n0:n0 + ns], start=False, stop=True)
                # relu (+ per-stripe sum) into staging, then spill to DRAM
                zs = wrk.tile([d_out, STRIPE], f32, name=f"zs{l}", tag="zs")
                nc.scalar.activation(out=zs[:, :ns], in_=ps[:, :ns],
                                     func=AT.Relu,
                                     accum_out=stats[:, s:s + 1])
                nc.scalar.activation(out=sq[:, :ns], in_=zs[:, :ns],
                                     func=AT.Square,
                                     accum_out=stats[:, nstripe + s:nstripe + s + 1])
                nc.sync.dma_start(zraw[:d_out, n0:n0 + ns], zs[:, :ns])
            # local sums
            loc_sums = wrk.tile([d_out, 2], f32, name=f"bnsum{l}")
            nc.vector.tensor_reduce(out=loc_sums[:, 0:1], in_=stats[:, :nstripe],
                                    axis=mybir.AxisListType.X, op=mybir.AluOpType.add)
            nc.vector.tensor_reduce(out=loc_sums[:, 1:2],
                                    in_=stats[:, nstripe:2 * nstripe],
                                    axis=mybir.AxisListType.X, op=mybir.AluOpType.add)
            if d_out < 128:
                pad = wrk.tile([128 - d_out, 2], f32, name=f"bnpad{l}")
                nc.vector.memset(pad[:], 0.0)
            nc.sync.dma_start(bn_in[:d_out, :], loc_sums[:])
            nc.gpsimd.collective_compute(
                "AllReduce", mybir.AluOpType.add,
                replica_groups=[list(range(NCORE))],
                ins=[bn_in[:].opt()], outs=[bn_out[:].opt()])
            gsum = wrk.tile([d_out, 2], f32, name=f"bng{l}")
            nc.sync.dma_start(gsum[:], bn_out[:d_out, :])
            # mu = gsum0/N ; ex2 = gsum1/N ; var = ex2 - mu^2
            mu = wrk.tile([d_out, 1], f32, name=f"mu{l}")
            var = wrk.tile([d_out, 1], f32, name=f"var{l}")
            nc.vector.tensor_scalar(out=mu[:], in0=gsum[:, 0:1], scalar1=1.0 / N,
                                    scalar2=None, op0=mybir.AluOpType.mult)
            nc.vector.tensor_scalar(out=var[:], in0=gsum[:, 1:2], scalar1=1.0 / N,
                                    scalar2=None, op0=mybir.AluOpType.mult)
            mu2 = wrk.tile([d_out, 1], f32, name=f"mu2{l}")
            nc.vector.tensor_tensor(out=mu2[:], in0=mu[:], in1=mu[:],
                                    op=mybir.AluOpType.mult)
            nc.vector.tensor_tensor(out=var[:], in0=var[:], in1=mu2[:],
                                    op=mybir.AluOpType.subtract)
            # rs = 1/sqrt(var+eps)
            sd = wrk.tile([d_out, 1], f32, name=f"sd{l}")
            nc.vector.tensor_scalar(out=sd[:], in0=var[:], scalar1=BN_EPS,
                                    scalar2=None, op0=mybir.AluOpType.add)
            nc.scalar.sqrt(sd[:], sd[:])
            rs = wrk.tile([d_out, 1], f32, name=f"rs{l}")
            nc.vector.reciprocal(rs[:], sd[:])
            # scale = gamma*rs ; bias = beta - mu*scale
            scale = wrk.tile([d_out, 1], f32, name=f"scale{l}")
            nc.vector.tensor_tensor(out=scale[:], in0=gb_sb[l][:, 0:1], in1=rs[:],
                                    op=mybir.AluOpType.mult)
            bias = wrk.tile([d_out, 1], f32, name=f"bias{l}")
            nc.vector.tensor_tensor(out=bias[:], in0=mu[:], in1=scale[:],
                                    op=mybir.AluOpType.mult)
            nc.vector.tensor_tensor(out=bias[:], in0=gb_sb[l][:, 1:2], in1=bias[:],
                                    op=mybir.AluOpType.subtract)
            return scale, bias

        # ============== per-layer driver ==============
        x_nm = [xs_d, hslice[0], hslice[1]]       # node-major per-core rows
        gather_src = [xp_d, hfull[0], hfull[1]]   # full-table gather sources
        pool_sb = wrk.tile([NUM_GRAPHS, 128], f32, name="pool_sb")
        nc.vector.memset(pool_sb[:], 0.0)
        for l in range(3):
            d_in, d_out = dims[l], dims[l + 1]
            agg_in = big.tile([d_in, S], f32, name=f"aggin{l}", tag="bigbuf")
            scatter_direction(l, s_in, idx_d["in"], loc_d["in"],
                              gather_src[l], d_in, agg_in)
            agg_out = big.tile([d_in, S], f32, name=f"aggout{l}", tag="bigbuf")
            scatter_direction(l, s_out, idx_d["out"], loc_d["out"],
                              gather_src[l], d_in, agg_out)
            scale, bias = dense_layer(l, d_in, d_out, x_nm[l], agg_in, agg_out)

            # h-pass: load zraw stripes, apply BN, transpose to node-major;
            # layers 0-1 write hslice + AllGather, layer 2 feeds pooling.
            nstripe = (S + STRIPE - 1) // STRIPE
            for sp in range(nstripe):
                n0 = sp * STRIPE
                ns = min(STRIPE, S - n0)
                hload = wrk.tile([d_out, STRIPE], f32, name=f"hload{l}", tag="hload")
                nc.sync.dma_start(hload[:, :ns], zraw[:d_out, n0:n0 + ns])
                nc.scalar.activation(out=hload[:, :ns], in_=hload[:, :ns],
                                     func=AT.Identity, scale=scale[:], bias=bias[:])
                for t0 in range(0, ns, 128):
                    tn = min(128, ns - t0)
                    t = (n0 + t0) // 128
                    tp = tpps.tile([128, 128], f32, space="PSUM", name=f"htp{l}", tag="tp")
                    nc.tensor.transpose(out=tp[:tn, :], in_=hload[:, t0:t0 + tn],
                                        identity=ident[:])
                    hst = stg.tile([128, 128], f32, name=f"hst{l}", tag="hst")
                    nc.scalar.copy(hst[:tn, :], tp[:tn, :])
                    if l < 2:
                        nc.sync.dma_start(hslice[l][n0 + t0:n0 + t0 + tn, :],
                                          hst[:tn, :])
                    else:
                        goh = stg.tile([128, NUM_GRAPHS], f32, name="goh", tag="goh")
                        nc.vector.tensor_scalar(
                            out=goh[:tn, :], in0=iota_g[:tn, :],
                            scalar1=bloc_sb[:tn, t:t + 1], scalar2=None,
                            op0=mybir.AluOpType.is_equal)
                        pool_ps = plps.tile([NUM_GRAPHS, 128], f32,
                                            space="PSUM", name="poolps")
                        nc.tensor.matmul(out=pool_ps[:], lhsT=goh[:tn, :],
                                         rhs=hst[:tn, :], start=True, stop=True)
                        nc.vector.tensor_tensor(out=pool_sb[:], in0=pool_sb[:],
                                                in1=pool_ps[:],
                                                op=mybir.AluOpType.add)
            if l < 2:
                nc.gpsimd.collective_compute(
                    "AllGather", mybir.AluOpType.bypass,
                    replica_groups=[list(range(NCORE))],
                    ins=[hslice[l][:].opt()], outs=[hfull[l][:].opt()])

        # ============== pooling reduce ==============
        nc.sync.dma_start(pool_in[:], pool_sb[:])
        nc.gpsimd.collective_compute(
            "AllReduce", mybir.AluOpType.add,
            replica_groups=[list(range(NCORE))],
            ins=[pool_in[:].opt()], outs=[pool_out[:].opt()])
        pool_g = wrk.tile([NUM_GRAPHS, 128], f32, name="pool_g")
        nc.sync.dma_start(pool_g[:], pool_out[:])
        res = wrk.tile([NUM_GRAPHS, 128], f32, name="res")
        nc.vector.tensor_scalar(out=res[:], in0=pool_g[:], scalar1=recip_sb[:],
                                scalar2=None, op0=mybir.AluOpType.mult)
        nc.sync.dma_start(out_d[:], res[:])
        ctx.close()


# ----------------------------------------------------------------------------
# public entry
# ----------------------------------------------------------------------------

_CACHE = {}


def _pjrt_runner(nc, in_maps, out_names_shapes):
    """Build a reusable sharded-jit executable for the SPMD program."""
    import jax
    from jax.sharding import Mesh, PartitionSpec, NamedSharding
    from jax.experimental.shard_map import shard_map
    from concourse import bass2jax

    bass2jax.install_neuronx_cc_hook()
    in_names = list(in_maps[0].keys())
    out_names = [n for n, _ in out_names_shapes]
    out_avals = [jax.core.ShapedArray(s, np.float32)
                 for _, s in out_names_shapes]
    pname = nc.partition_id_tensor.name if nc.partition_id_tensor else None
    all_in = in_names + out_names + ([pname] if pname else [])

    def _body(*args):
        ops = list(args)
        if pname is not None:
            ops.append(bass2jax.partition_id_tensor())
        return tuple(bass2jax._bass_exec_p.bind(
            *ops, out_avals=tuple(out_avals), in_names=tuple(all_in),
            out_names=tuple(out_names),
            lowering_input_output_aliases=(), sim_require_finite=True,
            sim_require_nnan=True, nc=nc))

    devices = jax.devices()[:NCORE]
    mesh = Mesh(np.asarray(devices), ("core",))
    nin = len(in_names) + len(out_names)
    fn = jax.jit(shard_map(_body, mesh=mesh,
                           in_specs=(PartitionSpec("core"),) * nin,
                           out_specs=(PartitionSpec("core"),) * len(out_names),
                           check_rep=False), keep_unused=True)
    sh = NamedSharding(mesh, PartitionSpec("core"))
    dev_in = [jax.device_put(
        np.concatenate([np.asarray(m[n]) for m in in_maps], axis=0), sh)
        for n in in_names]
    dev_zero = [jax.device_put(
        np.zeros((NCORE * s[0],) + tuple(s[1:]), np.float32), sh)
        for _, s in out_names_shapes]

    def run():
        outs = fn(*dev_in, *dev_zero)
        jax.block_until_ready(outs)
        return {n: np.asarray(outs[i]).reshape((NCORE,) + tuple(out_avals[i].shape))
                for i, n in enumerate(out_names)}

    return run


def _make_runner(x, edge_index, batch, weights):
    import concourse.bacc as bacc
    import concourse.bass as bass
    import concourse.tile as tile
    from concourse import mybir
    from concourse.bass_utils import run_bass_kernel_spmd

    N, Din = x.shape
    S = N // NCORE
    s_in, s_out, xs, bloc, recip = _preprocess(x, edge_index, batch)
    dims = [Din] + [weights[f"Ws{i}"].shape[1] for i in range(3)]

    nc = bacc.Bacc("TRN2", target_bir_lowering=False, debug=False,
                   num_devices=NCORE, num_swdge_queues=NQ)
    meta = dict(N=N, Din=Din, dims=dims, s_in=s_in, s_out=s_out)
    _build(nc, tile, mybir, bass, meta)
    nc.compile()

    in_maps = []
    for c in range(NCORE):
        m = {
            "xp": x,
            "xs": xs[c],
            "idx_in": s_in["idx_wrapped"][c],
            "loc_in": s_in["loc_tiled"][c],
            "idx_out": s_out["idx_wrapped"][c],
            "loc_out": s_out["loc_tiled"][c],
            "bloc": bloc[c],
            "recip": recip,
        }
        for i in range(3):
            m[f"ws{i}"] = weights[f"Ws{i}"]
            m[f"wi{i}"] = weights[f"Wi{i}"]
            m[f"wo{i}"] = weights[f"Wo{i}"]
            m[f"gb{i}"] = np.stack(
                [weights[f"g{i}"], weights[f"b{i}"]], axis=1).astype(np.float32)
        in_maps.append(m)

    import os
    if os.environ.get("KERNEL_SIM"):
        from concourse.bass_interp import MultiCoreSim

        def run():
            sim = MultiCoreSim(nc, num_cores=NCORE, trace=False,
                               require_finite=False, require_nnan=False)
            for c, core in sim.cores.items():
                for k, v in in_maps[c].items():
                    core.tensor(k)[:] = v
            sim.simulate(check_with_hw=False)

            class R:
                results = [{"out": np.array(sim.cores[c].tensor("out"))}
                           for c in range(NCORE)]
            return R()

        return run

    runner = _pjrt_runner(nc, in_maps, [("out", (NUM_GRAPHS, 128))])

    def run():
        outs = runner()

        class R:
            results = [{"out": outs["out"][c]} for c in range(NCORE)]
        return R()

    return run


def kernel(x, edge_index, batch, **weights):
    x = np.asarray(x, dtype=np.float32)
    edge_index = np.asarray(edge_index)
    batch = np.asarray(batch)
    weights = {k: np.asarray(v, dtype=np.float32) for k, v in weights.items()}
    run = _make_runner(x, edge_index, batch, weights)
    res = run()
    return np.asarray(res.results[0]["out"], dtype=np.float32)
